# revision 55
# baseline (speedup 1.0000x reference)
"""Trainium2 Bass kernel for nn_BindingConstraintsNN (gnn_message_passing).

Fragment-parallel across 8 NeuronCores: each core owns 125 whole fragments
(12500 nodes).  Per outer iteration the line-search objective is evaluated
exactly as a quartic polynomial in alpha whose coefficients are global sums
-> one tiny AllGather per iteration instead of recomputing matmuls.

Self-contained: hardcodes N=100000, DL=256, F=100, NFRAG=1000, 8 cores.
"""

import os

os.environ.setdefault("NEURON_RT_RESET_CORES", "1")  # recover wedged cores

import numpy as np

import concourse.bass as bass
import concourse.bacc as bacc
import concourse.tile as tile
import concourse.mybir as mybir
from concourse import bass_utils

F32 = mybir.dt.float32
F32R = mybir.dt.float32r
BF16 = mybir.dt.bfloat16
ALU = mybir.AluOpType
AFT = mybir.ActivationFunctionType
AXL = mybir.AxisListType

D = 3.8
N_ITER = 10
CONVERGED = 1e-4
MAX_LS = 10
NCAND = MAX_LS + 2  # 12: alpha*2^-k, k=0..10 evaluated, 2^-11 fallback


def build_program(ncores, fpc, F, DL, a2=None, niter=N_ITER, use_cc=True):
    """Build (unscheduled) Bacc program for one core (SPMD across ncores).

    a2: optional 3x3 np array baked into the program as immediates (the
    dgx mixing matrix); falls back to the a2rep input tile when None.
    """
    E = F - 1
    NPC = fpc * F
    d2 = float(np.float32(D * D))  # match reference: jnp.float32(D*D)
    hch = [(s, min(128, DL - s)) for s in range(0, DL, 128)]
    nh = len(hch)
    hdim = hch[0][1]
    NE = NCAND - 1  # 11 evaluated candidates

    nc = bacc.Bacc("TRN2", target_bir_lowering=False, debug=False,
                   enable_asserts=True, num_devices=ncores)

    y_in = nc.dram_tensor("y", [NPC, DL], F32R, kind="ExternalInput")
    wp3t_in = nc.dram_tensor("wp3t", [hdim, 3 * nh], F32R,
                             kind="ExternalInput")
    ident_in = nc.dram_tensor("ident", [fpc, fpc], F32R,
                              kind="ExternalInput")
    weffnt_in = nc.dram_tensor("weffnegt", [3, DL], F32R,
                               kind="ExternalInput")
    a2rep_in = nc.dram_tensor("a2rep", [fpc, 9], F32, kind="ExternalInput")
    onescol_in = nc.dram_tensor("onescol", [fpc, 1], F32, kind="ExternalInput")
    onesg_in = nc.dram_tensor("onesg", [ncores, 1], F32, kind="ExternalInput")
    onesrow_in = nc.dram_tensor("onesrow", [1, fpc], F32, kind="ExternalInput")
    pw2e_in = nc.dram_tensor("pw2e", [1, NCAND], F32, kind="ExternalInput")
    pw2sel_in = nc.dram_tensor("pw2sel", [1, NCAND], F32,
                               kind="ExternalInput")
    kvec_in = nc.dram_tensor("kvec", [1, 5], F32, kind="ExternalInput")
    k4_in = nc.dram_tensor("k4", [1, 4], F32, kind="ExternalInput")
    wb_in = nc.dram_tensor("wb", [1, 6], F32, kind="ExternalInput")
    yout = nc.dram_tensor("yout", [NPC, DL], F32, kind="ExternalOutput")

    with tile.TileContext(nc) as tc:
        with tc.tile_pool(name="persist", bufs=1) as P1, \
             tc.tile_pool(name="dram", bufs=1, space="DRAM") as PD:

            # ---------------- constants into SBUF ----------------
            def const_tile(shape, src, tag, dtype=F32):
                t = P1.tile(shape, dtype, tag=tag)
                nc.sync.dma_start(t[:], src.ap())
                return t
            wp3t = const_tile([hdim, 3 * nh], wp3t_in, "wp3t", F32R)
            ident = const_tile([fpc, fpc], ident_in, "ident", F32R)
            weffnt = const_tile([3, DL], weffnt_in, "weffnt", F32R)
            a2s = const_tile([fpc, 9], a2rep_in, "a2s")
            onescol = const_tile([fpc, 1], onescol_in, "onescol")
            onesg = const_tile([ncores, 1], onesg_in, "onesg")
            onesrow = const_tile([1, fpc], onesrow_in, "onesrow")
            pw2e = const_tile([1, NCAND], pw2e_in, "pw2e")
            pw2sel = const_tile([1, NCAND], pw2sel_in, "pw2sel")
            kvec = const_tile([1, 5], kvec_in, "kvec")
            k4 = const_tile([1, 4], k4_in, "k4")
            wb = const_tile([1, 6], wb_in, "wb")

            # warm up the collective path: first AllGather pays ~55us
            if use_cc:
                with tc.tile_pool(name="warmcc", bufs=1, space="DRAM") as PW:
                    win = PW.tile([1, 4], F32, tag="win")
                    wout = PW.tile([ncores, 4], F32, tag="wout")
                    wsb = P1.tile([1, 4], F32, tag="wsb")
                    nc.vector.memset(wsb[:], 0.0)
                    nc.sync.dma_start(win[:], wsb[:])
                    nc.gpsimd.collective_compute(
                        "AllGather", ALU.bypass,
                        replica_groups=[list(range(ncores))],
                        ins=[win[:].opt()], outs=[wout[:].opt()])
                    win2 = PW.tile([1, 4], F32, tag="win2")
                    wout2 = PW.tile([ncores, 4], F32, tag="wout2")
                    nc.sync.dma_start(win2[:], wsb[:])
                    nc.gpsimd.collective_compute(
                        "AllGather", ALU.bypass,
                        replica_groups=[list(range(ncores))],
                        ins=[win2[:].opt()], outs=[wout2[:].opt()])

            # warm the ACT sqrt table set early (overlaps DMA)
            warm = P1.tile([1, 1], F32)
            nc.vector.memset(warm[:], 1.0)
            nc.scalar.activation(warm[:], warm[:], AFT.Sqrt)

            # -------- y load (4 sub-tiles, SWDGE = all 16 engines) --------
            if F % 20 == 0:
                ybnd = [0, F // 10, F // 5, 3 * F // 5, F]
            else:
                ybnd = [0, F // 2, F]
            y_ap = y_in.ap().rearrange("(p i) d -> p (i d)", p=fpc)
            ynat = []
            for ci in range(len(ybnd) - 1):
                lo_i, hi_i = ybnd[ci], ybnd[ci + 1]
                t = P1.tile([fpc, (hi_i - lo_i) * DL], F32R, tag=f"ynat{ci}")
                nc.gpsimd.dma_start(
                    t[:], y_ap[:, lo_i * DL:hi_i * DL])
                ynat.append(t)

            def ynat_slice(i, lo, w):
                for ci in range(len(ybnd) - 1):
                    if i < ybnd[ci + 1]:
                        off = (i - ybnd[ci]) * DL + lo
                        return ynat[ci][:, off:off + w]
                raise AssertionError

            def ynat_slice2(i):
                # [fpc, 2*DL] covering nodes i, i+1 (must be in same region)
                for ci in range(len(ybnd) - 1):
                    if i < ybnd[ci + 1]:
                        assert i + 1 < ybnd[ci + 1] or ci == len(ybnd) - 2
                        assert i + 1 <= ybnd[ci + 1]
                        off = (i - ybnd[ci]) * DL
                        return ynat[ci][:, off:off + 2 * DL]
                raise AssertionError

            # ---------------- loop-state tiles ----------------
            dx = P1.tile([fpc, 3 * E], F32)       # dx planes [fpc,3,E]
            cqr = P1.tile([fpc, 3 * E], F32)      # [c | q | r]
            c_t = cqr[:, 0:E]
            q_t = cqr[:, E:2 * E]
            r_t = cqr[:, 2 * E:3 * E]
            cdxp = P1.tile([fpc, 3 * (F + 1)], F32)   # padded [fpc,3,F+1]
            lam = P1.tile([fpc, 3 * F], F32)
            dlam = P1.tile([fpc, 3 * E], F32)
            dgx = P1.tile([fpc, 3 * E], F32)
            qp = P1.tile([fpc, 3 * E], F32)
            prod5 = P1.tile([fpc, 5 * E], F32)
            S_t = P1.tile([fpc, 3 * F], F32)
            scr2 = P1.tile([fpc, F], F32)
            stage_t = P1.tile([fpc, 16], F32)
            g8row = P1.tile([1, ncores * 16], F32)
            x3T = P1.tile([3, NPC], F32, tag="big3")
            x3p = P1.tile([fpc, 3 * F], F32)

            # scalar-stage tiles (partition 0)
            alpha = P1.tile([1, 1], F32)
            done = P1.tile([1, 1], F32)
            S0 = P1.tile([1, 1], F32)
            av = P1.tile([1, NCAND], F32)
            av2 = P1.tile([1, NCAND], F32)
            av3 = P1.tile([1, NCAND], F32)
            av4 = P1.tile([1, NCAND], F32)
            avsel = P1.tile([1, NCAND], F32)
            t5 = P1.tile([1, 5], F32)
            ct = P1.tile([1, NCAND], F32)
            okx = P1.tile([1, NCAND], F32)
            t12 = P1.tile([1, NCAND], F32)
            selv = P1.tile([1, NCAND], F32)
            selc = P1.tile([1, NCAND], F32)
            af = P1.tile([1, 1], F32)
            ctf = P1.tile([1, 1], F32)
            sA = P1.tile([1, 1], F32)
            sB = P1.tile([1, 1], F32)
            sC = P1.tile([1, 1], F32)
            u_t = P1.tile([1, 1], F32)
            sreg = P1.tile([1, 4], F32)
            pgs = P1.tile([1, 16], F32)
            s16 = P1.tile([1, 16], F32)
            bs_sb = P1.tile([fpc, 4], F32)
            S_scr = P1.tile([fpc, 3 * F], F32)
            sGA = P1.tile([1, 1], F32)
            sGB = P1.tile([1, 1], F32)
            sD = P1.tile([1, 1], F32)
            one1 = P1.tile([1, 1], F32)
            convc = P1.tile([1, 1], F32)
            Sbf = P1.tile([fpc, 3 * F], BF16)
            weffbf = P1.tile([3, DL], BF16)

            nc.vector.memset(cdxp[:], 0.0)
            nc.vector.memset(S_t[:], 0.0)
            nc.vector.memset(done[:], 0.0)
            nc.vector.memset(u_t[:], 1.0)
            nc.vector.memset(one1[:], 1.0)
            nc.vector.memset(convc[:], CONVERGED)
            nc.vector.memset(stage_t[:], 0.0)
            nc.vector.memset(okx[:, NCAND - 1:NCAND], 1.0)

            # 3d views
            dx3 = dx[:].rearrange("p (c e) -> p c e", c=3)
            cdxp3 = cdxp[:].rearrange("p (c e) -> p c e", c=3)
            lam3 = lam[:].rearrange("p (c e) -> p c e", c=3)
            dlam3 = dlam[:].rearrange("p (c e) -> p c e", c=3)
            dgx3 = dgx[:].rearrange("p (c e) -> p c e", c=3)
            qp3 = qp[:].rearrange("p (c e) -> p c e", c=3)
            x3p3 = x3p[:].rearrange("p (c e) -> p c e", c=3)
            x3T3 = x3T[:].rearrange("c (p f) -> c p f", f=F)

            # ---------------- Phase A: x3 = y @ Wp3.T ----------------
            # fp32r transposes (1.5 c/r) + 512-wide fp32r matmuls (1 c/r).
            IB = 4 if F % 4 == 0 else 2  # i's per psum bank
            with tc.tile_pool(name="psA", bufs=4, space="PSUM") as PSA, \
                 tc.tile_pool(name="psX", bufs=2, space="PSUM") as PSX, \
                 tc.tile_pool(name="yt", bufs=3) as PYT:
                for g in range(F // IB):
                    psx = PSX.tile([3, IB * fpc], F32, tag="psx")
                    yt4 = PYT.tile([hdim, nh * IB * fpc], F32R, tag="yt4")
                    yt4v = yt4[:].rearrange("p (h f) -> p h f", h=nh)
                    for i2 in range(IB):
                        i = g * IB + i2
                        pst = PSA.tile([hdim, nh * fpc], F32R, tag="pst")
                        for h, (lo, w) in enumerate(hch):
                            nc.tensor.transpose(
                                pst[:w, h * fpc:(h + 1) * fpc],
                                ynat_slice(i, lo, w),
                                ident[:])
                        dst = yt4v[:, :, i2 * fpc:(i2 + 1) * fpc]
                        src = pst[:].rearrange("p (h f) -> p h f", h=nh)
                        nc.vector.tensor_copy(dst, src)
                    for h in range(nh):
                        nc.tensor.matmul(
                            psx[:],
                            lhsT=wp3t[:, 3 * h:3 * h + 3],
                            rhs=yt4[:, h * IB * fpc:(h + 1) * IB * fpc],
                            start=(h == 0), stop=(h == nh - 1))
                    # drain bank -> x3T columns {p*F + g*IB + i2} (ACT)
                    src = psx[:].rearrange("c (d p) -> c p d", p=fpc)
                    dst = x3T3[:, :, g * IB:(g + 1) * IB]
                    nc.scalar.activation(dst, src, AFT.Copy)

            # scatter x3T -> fragment-major planes
            for j in range(3):
                nc.sync.dma_start(x3p[:, j * F:(j + 1) * F], x3T[j:j + 1, :])
            # dx = diff(x3) ; c = |dx|^2 - d2
            nc.vector.tensor_tensor(out=dx3[:, :, :], in0=x3p3[:, :, 1:F],
                                    in1=x3p3[:, :, 0:E], op=ALU.subtract)
            nc.vector.tensor_tensor(out=qp[:], in0=dx[:], in1=dx[:],
                                    op=ALU.mult)
            nc.vector.tensor_tensor(out=c_t, in0=qp3[:, 0, :],
                                    in1=qp3[:, 1, :], op=ALU.add)
            nc.vector.tensor_tensor(out=c_t, in0=c_t, in1=qp3[:, 2, :],
                                    op=ALU.add)
            nc.vector.tensor_scalar_add(out=c_t, in0=c_t, scalar1=-d2)

            # ---------------- Phase B: constraint iterations ----------------
            with tc.tile_pool(name="psS", bufs=1, space="PSUM") as PSS, \
                 tc.tile_pool(name="psB", bufs=1, space="PSUM") as PSB, \
                 tc.tile_pool(name="ccd", bufs=2, space="DRAM") as PCD:

                for it in range(niter):
                    first = (it == 0)
                    # cdx = c * dx  (into padded buffer cols 1..F-1)
                    nc.vector.tensor_tensor(
                        out=cdxp3[:, :, 1:F], in0=dx3[:, :, :],
                        in1=c_t.unsqueeze(1).broadcast_to((fpc, 3, E)),
                        op=ALU.mult)
                    # dlam[e] = 2*cdxp[e+1] - cdxp[e] - cdxp[e+2]
                    nc.vector.scalar_tensor_tensor(
                        out=dlam3[:, :, :], in0=cdxp3[:, :, 1:F], scalar=2.0,
                        in1=cdxp3[:, :, 0:E], op0=ALU.mult, op1=ALU.subtract)
                    nc.vector.tensor_tensor(
                        out=dlam3[:, :, :], in0=dlam3[:, :, :],
                        in1=cdxp3[:, :, 2:F + 1], op=ALU.subtract)
                    if first:  # lam needed pre-AG only for the lamB sums
                        nc.vector.tensor_tensor(
                            out=lam3[:, :, :], in0=cdxp3[:, :, 0:F],
                            in1=cdxp3[:, :, 1:F + 1], op=ALU.subtract)
                    # dgx_j = sum_k A2[j,k] dlam_k
                    for j in range(3):
                        if a2 is not None:  # immediates: no SBUF ptr reads
                            nc.vector.tensor_scalar_mul(
                                out=dgx3[:, j, :], in0=dlam3[:, 0, :],
                                scalar1=float(a2[j, 0]))
                            for k in (1, 2):
                                nc.vector.scalar_tensor_tensor(
                                    out=dgx3[:, j, :], in0=dlam3[:, k, :],
                                    scalar=float(a2[j, k]),
                                    in1=dgx3[:, j, :],
                                    op0=ALU.mult, op1=ALU.add)
                        else:
                            nc.vector.tensor_scalar_mul(
                                out=dgx3[:, j, :], in0=dlam3[:, 0, :],
                                scalar1=a2s[:, 3 * j:3 * j + 1])
                            for k in (1, 2):
                                nc.vector.scalar_tensor_tensor(
                                    out=dgx3[:, j, :], in0=dlam3[:, k, :],
                                    scalar=a2s[:, 3 * j + k:3 * j + k + 1],
                                    in1=dgx3[:, j, :],
                                    op0=ALU.mult, op1=ALU.add)
                    # q = <dx, dgx> on DVE; r = <dgx, dgx> on GpSimd
                    nc.gpsimd.tensor_tensor(out=S_scr[:, 0:3 * E],
                                            in0=dgx[:], in1=dgx[:],
                                            op=ALU.mult)
                    nc.vector.tensor_tensor(out=qp[:], in0=dx[:], in1=dgx[:],
                                            op=ALU.mult)
                    nc.vector.tensor_tensor(out=q_t, in0=qp3[:, 0, :],
                                            in1=qp3[:, 1, :], op=ALU.add)
                    nc.vector.tensor_tensor(out=q_t, in0=q_t,
                                            in1=qp3[:, 2, :], op=ALU.add)
                    nc.gpsimd.tensor_tensor(out=r_t, in0=S_scr[:, 0:E],
                                            in1=S_scr[:, E:2 * E],
                                            op=ALU.add)
                    nc.gpsimd.tensor_tensor(out=r_t, in0=r_t,
                                            in1=S_scr[:, 2 * E:3 * E],
                                            op=ALU.add)

                    # partial sums: prod5 = [cq, q2, qr, r2, cr]
                    nc.vector.tensor_tensor(out=prod5[:, 0:E],
                                            in0=c_t, in1=q_t, op=ALU.mult)
                    nc.vector.tensor_tensor(out=prod5[:, E:2 * E],
                                            in0=q_t, in1=q_t, op=ALU.mult)
                    nc.vector.tensor_tensor(out=prod5[:, 2 * E:3 * E],
                                            in0=q_t, in1=r_t, op=ALU.mult)
                    nc.vector.tensor_tensor(out=prod5[:, 3 * E:4 * E],
                                            in0=r_t, in1=r_t, op=ALU.mult)
                    nc.vector.tensor_tensor(out=prod5[:, 4 * E:5 * E],
                                            in0=c_t, in1=r_t, op=ALU.mult)
                    nc.vector.tensor_reduce(
                        out=stage_t[:, 0:5],
                        in_=prod5[:].rearrange("p (g e) -> p g e", g=5),
                        axis=AXL.X, op=ALU.add)
                    if first:
                        nc.vector.tensor_tensor(out=prod5[:, 0:E], in0=c_t,
                                                in1=c_t, op=ALU.mult)
                        nc.vector.tensor_reduce(
                            out=stage_t[:, 5:6], in_=prod5[:, 0:E],
                            axis=AXL.X, op=ALU.add)
                        for col, (ja, jb) in enumerate(
                                [(0, 0), (1, 1), (2, 2), (0, 1), (0, 2),
                                 (1, 2)]):
                            nc.vector.tensor_tensor(
                                out=scr2[:], in0=lam3[:, ja, :],
                                in1=lam3[:, jb, :], op=ALU.mult)
                            nc.vector.tensor_reduce(
                                out=stage_t[:, 6 + col:7 + col], in_=scr2[:],
                                axis=AXL.X, op=ALU.add)

                    # hole-fillers: run during the collective round-trip
                    if not first:
                        nc.gpsimd.tensor_tensor(
                            out=lam3[:, :, :], in0=cdxp3[:, :, 0:F],
                            in1=cdxp3[:, :, 1:F + 1], op=ALU.subtract)
                    if not first:  # iter 0: alpha not yet known
                        nc.vector.tensor_scalar_mul(out=av[:], in0=pw2e[:],
                                                    scalar1=alpha[:])
                        nc.vector.tensor_scalar_mul(out=avsel[:],
                                                    in0=pw2sel[:],
                                                    scalar1=alpha[:])
                        nc.vector.tensor_tensor(out=av2[:], in0=av[:],
                                                in1=av[:], op=ALU.mult)
                        nc.vector.tensor_tensor(out=av3[:], in0=av2[:],
                                                in1=av[:], op=ALU.mult)
                        nc.vector.tensor_tensor(out=av4[:], in0=av2[:],
                                                in1=av2[:], op=ALU.mult)
                        # u = 1 - done (prev iteration's done)
                        nc.gpsimd.tensor_tensor(out=u_t[:], in0=one1[:],
                                                in1=done[:], op=ALU.subtract)

                    # partition-reduce on PE, bounce, allgather
                    ps1 = PSS.tile([1, 16], F32, tag="ps1")
                    nc.tensor.matmul(ps1[:], lhsT=onescol[:], rhs=stage_t[:],
                                     start=True, stop=True)
                    nc.vector.tensor_copy(s16[:], ps1[:])
                    cc_in = PCD.tile([1, 16], F32, tag="cci")
                    cc_out = PCD.tile([ncores, 16], F32, tag="cco")
                    nc.sync.dma_start(cc_in[:], s16[:])
                    if use_cc:
                        nc.gpsimd.collective_compute(
                            "AllGather", ALU.bypass,
                            replica_groups=[list(range(ncores))],
                            ins=[cc_in[:].opt()], outs=[cc_out[:].opt()])
                    else:  # debug: replicate local sums
                        nc.sync.dma_start(
                            cc_out[:],
                            cc_in[0:1, :].broadcast_to((ncores, 16)))
                    nc.sync.dma_start(g8row[:], cc_out[:])
                    nc.vector.tensor_reduce(
                        out=pgs[:],
                        in_=g8row[:].rearrange("o (r c) -> o c r", c=16),
                        axis=AXL.X, op=ALU.add)

                    # ---------------- scalar stage ----------------
                    if first:
                        # alpha = 1/sqrt(dot(wb, pgs[6:12])), newton-polished
                        nc.vector.tensor_tensor(out=t12[:, 0:6],
                                                in0=pgs[:, 6:12], in1=wb[:],
                                                op=ALU.mult)
                        nc.vector.tensor_reduce(out=sA[:], in_=t12[:, 0:6],
                                                axis=AXL.X, op=ALU.add)
                        nc.scalar.activation(sB[:], sA[:], AFT.Sqrt)
                        nc.vector.reciprocal(alpha[:], sB[:])
                        nc.vector.tensor_tensor(out=sB[:], in0=alpha[:],
                                                in1=alpha[:], op=ALU.mult)
                        nc.vector.tensor_scalar(out=sB[:], in0=sB[:],
                                                scalar1=sA[:], scalar2=-0.5,
                                                op0=ALU.mult, op1=ALU.mult)
                        nc.vector.tensor_scalar_add(out=sB[:], in0=sB[:],
                                                    scalar1=1.5)
                        nc.vector.tensor_tensor(out=alpha[:], in0=alpha[:],
                                                in1=sB[:], op=ALU.mult)
                        nc.vector.tensor_copy(S0[:], pgs[:, 5:6])
                        nc.vector.tensor_scalar_mul(out=av[:], in0=pw2e[:],
                                                    scalar1=alpha[:])
                        nc.vector.tensor_scalar_mul(out=avsel[:],
                                                    in0=pw2sel[:],
                                                    scalar1=alpha[:])
                        nc.vector.tensor_tensor(out=av2[:], in0=av[:],
                                                in1=av[:], op=ALU.mult)
                        nc.vector.tensor_tensor(out=av3[:], in0=av2[:],
                                                in1=av[:], op=ALU.mult)
                        nc.vector.tensor_tensor(out=av4[:], in0=av2[:],
                                                in1=av2[:], op=ALU.mult)
                    # t5 = kvec * [Scq, Sq2, Sqr, Sr2, Scr]
                    #    = [-4Scq, 4Sq2, -4Sqr, Sr2, 2Scr]
                    nc.vector.tensor_tensor(out=t5[:], in0=pgs[:, 0:5],
                                            in1=kvec[:], op=ALU.mult)
                    nc.vector.scalar_tensor_tensor(
                        out=ct[:], in0=av[:], scalar=t5[:, 0:1],
                        in1=S0[:].broadcast_to((1, NCAND)),
                        op0=ALU.mult, op1=ALU.add)
                    for pw, colx in ((av2, 1), (av2, 4), (av3, 2), (av4, 3)):
                        nc.vector.scalar_tensor_tensor(
                            out=ct[:], in0=pw[:],
                            scalar=t5[:, colx:colx + 1], in1=ct[:],
                            op0=ALU.mult, op1=ALU.add)
                    # ok[k] = ct[k] < cnorm ; col NCAND-1 preset to 1
                    nc.vector.tensor_scalar(out=okx[:, 0:NE], in0=ct[:, 0:NE],
                                            scalar1=S0[:], scalar2=None,
                                            op0=ALU.is_lt)
                    # first success (av strictly decreasing): af = max(ok*av)
                    nc.vector.tensor_tensor(out=t12[:], in0=okx[:],
                                            in1=avsel[:], op=ALU.mult)
                    nc.vector.tensor_reduce(out=af[:], in_=t12[:],
                                            axis=AXL.X, op=ALU.max)
                    # ctf = ct at the selected candidate (unique max match)
                    nc.vector.tensor_scalar(out=selv[:], in0=t12[:],
                                            scalar1=af[:], scalar2=None,
                                            op0=ALU.is_equal)
                    nc.vector.tensor_tensor(out=selc[:], in0=selv[:],
                                            in1=ct[:], op=ALU.mult)
                    nc.vector.tensor_reduce(out=ctf[:], in_=selc[:],
                                            axis=AXL.X, op=ALU.add)
                    # growth: af *= 1 + 0.5*ok0*(ctf > CONVERGED)
                    nc.vector.tensor_scalar(out=sA[:], in0=ctf[:],
                                            scalar1=CONVERGED, scalar2=None,
                                            op0=ALU.is_gt)
                    nc.vector.tensor_scalar(out=sA[:], in0=sA[:],
                                            scalar1=okx[:, 0:1], scalar2=0.5,
                                            op0=ALU.mult, op1=ALU.mult)
                    nc.vector.scalar_tensor_tensor(
                        out=af[:], in0=sA[:], scalar=1.0, in1=af[:],
                        op0=ALU.add, op1=ALU.mult)
                    # a_eff = af*u  (u = 1-done from prev iteration)
                    nc.vector.tensor_tensor(out=sC[:], in0=af[:], in1=u_t[:],
                                            op=ALU.mult)
                    # sreg = [-a, -2a, a^2, 2a]
                    nc.vector.tensor_scalar_mul(out=sreg[:], in0=k4[:],
                                                scalar1=sC[:])
                    nc.vector.tensor_tensor(out=sreg[:, 2:3], in0=sC[:],
                                            in1=sC[:], op=ALU.mult)

                    # broadcast sreg via PE, apply updates
                    bs = PSB.tile([fpc, 4], F32, tag="bs")
                    nc.tensor.matmul(bs[:], lhsT=onesrow[:], rhs=sreg[:],
                                     start=True, stop=True)
                    nc.vector.scalar_tensor_tensor(
                        out=dx[:], in0=dgx[:], scalar=bs[:, 0:1], in1=dx[:],
                        op0=ALU.mult, op1=ALU.add)
                    nc.vector.scalar_tensor_tensor(
                        out=c_t, in0=q_t, scalar=bs[:, 1:2], in1=c_t,
                        op0=ALU.mult, op1=ALU.add)
                    nc.vector.scalar_tensor_tensor(
                        out=c_t, in0=r_t, scalar=bs[:, 2:3], in1=c_t,
                        op0=ALU.mult, op1=ALU.add)
                    nc.scalar.activation(bs_sb[:], bs[:], AFT.Copy)
                    nc.gpsimd.tensor_tensor(
                        out=S_scr[:], in0=lam[:],
                        in1=bs_sb[:, 3:4].broadcast_to((fpc, 3 * F)),
                        op=ALU.mult)
                    nc.gpsimd.tensor_tensor(
                        out=S_t[:], in0=S_t[:], in1=S_scr[:], op=ALU.add)

                    # state upkeep on GpSimd, off the critical path (consumed
                    # only by the NEXT iteration's shadow / scalar stage)
                    nc.gpsimd.tensor_tensor(out=sGA[:], in0=af[:],
                                            in1=alpha[:], op=ALU.subtract)
                    nc.gpsimd.tensor_tensor(out=sGA[:], in0=sGA[:],
                                            in1=u_t[:], op=ALU.mult)
                    nc.gpsimd.tensor_tensor(out=alpha[:], in0=alpha[:],
                                            in1=sGA[:], op=ALU.add)
                    nc.gpsimd.tensor_tensor(out=sGB[:], in0=ctf[:],
                                            in1=S0[:], op=ALU.subtract)
                    nc.gpsimd.tensor_tensor(out=sGB[:], in0=sGB[:],
                                            in1=u_t[:], op=ALU.mult)
                    nc.gpsimd.tensor_tensor(out=S0[:], in0=S0[:],
                                            in1=sGB[:], op=ALU.add)
                    nc.vector.tensor_tensor(out=sD[:], in0=ctf[:],
                                            in1=convc[:], op=ALU.is_lt)
                    nc.vector.tensor_tensor(out=done[:], in0=done[:],
                                            in1=sD[:], op=ALU.max)

            # ---------------- Phase C: yout = y - S @ Weff.T ----------------
            # bf16 matmuls: 1 cycle/row at any p-state + fast weight load
            nc.vector.tensor_copy(weffbf[:], weffnt[:].bitcast(F32))
            nc.vector.tensor_copy(Sbf[:], S_t[:])
            ST = x3T[:, 0:NPC // 2].bitcast(BF16)  # reuse x3T's bytes
            S3 = Sbf[:].rearrange("p (c f) -> p c f", c=3)
            for j in range(3):
                nc.sync.dma_start(ST[j:j + 1, :], S3[:, j, :])
            ST3 = ST.rearrange("c (p f) -> c p f", f=F)
            OB = 10 if F % 10 == 0 else max(
                b for b in (5, 2, 1) if F % b == 0)  # i's per out block
            dst_y = yout.ap().rearrange("(p f) d -> p f d", p=fpc)
            with tc.tile_pool(name="psF", bufs=4, space="PSUM") as PSF, \
                 tc.tile_pool(name="obuf", bufs=2) as POB:
                for blk in range(F // OB):
                    ob = POB.tile([fpc, OB * DL], F32, tag="ob")
                    for i2 in range(0, OB, 2):
                        i = blk * OB + i2
                        bank = PSF.tile([fpc, 2 * DL], F32, tag="fin")
                        for t in range(2):
                            nc.tensor.matmul(
                                bank[:, t * DL:(t + 1) * DL],
                                lhsT=ST3[:, :, i + t],
                                rhs=weffbf[:],
                                start=True, stop=True)
                        pair_dst = ob[:, i2 * DL:(i2 + 2) * DL]
                        if (i2 // 2) % 5 != 2:  # 4/5 of pairs on DVE
                            nc.vector.tensor_tensor(
                                out=pair_dst, in0=bank[:],
                                in1=ynat_slice2(i).bitcast(F32), op=ALU.add)
                        else:  # 1/5 via ACT copy + GpSimd add
                            sc = POB.tile([fpc, 2 * DL], F32, tag="sc")
                            nc.scalar.activation(sc[:], bank[:], AFT.Copy)
                            nc.gpsimd.tensor_tensor(
                                out=pair_dst, in0=sc[:],
                                in1=ynat_slice2(i).bitcast(F32), op=ALU.add)
                    nc.gpsimd.dma_start(
                        dst_y[:, blk * OB:(blk + 1) * OB, :], ob[:])

    return nc


def make_consts(Wp, Wu, fpc, ncores, nreal=None):
    if nreal is None:
        nreal = fpc
    DL = Wp.shape[1]
    hch = [(s, min(128, DL - s)) for s in range(0, DL, 128)]
    nh = len(hch)
    hdim = hch[0][1]
    Wp3 = Wp[:3].astype(np.float32)
    Weff = (Wu[:, 0:3] + Wu[:, 3:6] + Wu[:, 6:9]).astype(np.float32)
    A2 = 2.0 * (Wp3 @ Weff)
    B = Weff.T @ Weff
    wb = 4.0 * np.array([[B[0, 0], B[1, 1], B[2, 2],
                          2 * B[0, 1], 2 * B[0, 2], 2 * B[1, 2]]], np.float32)
    wp3t = np.zeros((hdim, 3 * nh), np.float32)
    for h, (lo, w) in enumerate(hch):
        wp3t[:w, 3 * h:3 * h + 3] = Wp3[:, lo:lo + w].T
    pw2e = (2.0 ** -np.arange(NCAND, dtype=np.float32))
    pw2e[NCAND - 1] = pw2e[NCAND - 2]  # last candidate duplicates ct(2^-10)
    pw2sel = (2.0 ** -np.arange(NCAND, dtype=np.float32))
    return {
        "wp3t": wp3t,
        "ident": np.eye(fpc, dtype=np.float32),
        "weffnegt": np.ascontiguousarray(-Weff.T),
        "a2rep": np.tile(A2.reshape(1, 9), (fpc, 1)).astype(np.float32),
        "onescol": (np.arange(fpc) < nreal).astype(
            np.float32).reshape(fpc, 1),
        "onesg": np.ones((ncores, 1), np.float32),
        "onesrow": np.ones((1, fpc), np.float32),
        "pw2e": pw2e.reshape(1, NCAND),
        "pw2sel": pw2sel.reshape(1, NCAND),
        "kvec": np.array([[-4.0, 4.0, -4.0, 1.0, 2.0]], np.float32),
        "k4": np.array([[-1.0, -2.0, 0.0, 2.0]], np.float32),
        "wb": wb,
    }


_PROG_CACHE = {}


def _get_program(ncores, fpc, F, DL, a2):
    key = (ncores, fpc, F, DL, a2.tobytes())
    if key not in _PROG_CACHE:
        nc = build_program(ncores, fpc, F, DL, a2=a2)
        nc.compile()
        _PROG_CACHE[key] = nc
    return _PROG_CACHE[key]


def prepare(inputs):
    """Build/compile program and padded in_maps (shared with test harness)."""
    y = np.ascontiguousarray(np.asarray(inputs["y"], np.float32))
    Wp = np.asarray(inputs["Wp"], np.float32)
    Wu = np.asarray(inputs["Wu"], np.float32)
    N, DL = y.shape
    NCORES, F = 8, 100
    fpc = N // F // NCORES
    NPC = N // NCORES
    fpc_pad = 128
    NPC_pad = fpc_pad * F
    Wp3 = Wp[:3].astype(np.float32)
    Weff = (Wu[:, 0:3] + Wu[:, 3:6] + Wu[:, 6:9]).astype(np.float32)
    a2 = (2.0 * (Wp3 @ Weff)).astype(np.float32)
    nc = _get_program(NCORES, fpc_pad, F, DL, a2)
    consts = make_consts(Wp, Wu, fpc_pad, NCORES, nreal=fpc)
    in_maps = []
    for i in range(NCORES):
        sh = np.zeros((NPC_pad, DL), np.float32)
        sh[:NPC] = y[i * NPC:(i + 1) * NPC]
        in_maps.append({"y": sh, **consts})
    return nc, in_maps, NPC


def kernel(**inputs):
    y = np.ascontiguousarray(np.asarray(inputs["y"], np.float32))
    Wp = np.asarray(inputs["Wp"], np.float32)
    Wu = np.asarray(inputs["Wu"], np.float32)
    N, DL = y.shape
    NCORES, F = 8, 100
    NFRAG = N // F
    fpc = NFRAG // NCORES
    NPC = N // NCORES

    nc, in_maps, NPC_r = prepare(inputs)
    res = bass_utils.run_bass_kernel_spmd(
        nc, in_maps, core_ids=list(range(NCORES)))
    out = np.concatenate(
        [res.results[i]["yout"][:NPC_r] for i in range(NCORES)], axis=0)
    return out.astype(inputs["y"].dtype, copy=False)


# revision 57
# speedup vs baseline: 1.0286x; 1.0286x over previous
"""Trainium2 Bass kernel for nn_BindingConstraintsNN (gnn_message_passing).

Fragment-parallel across 8 NeuronCores: each core owns 125 whole fragments
(12500 nodes).  Per outer iteration the line-search objective is evaluated
exactly as a quartic polynomial in alpha whose coefficients are global sums
-> one tiny AllGather per iteration instead of recomputing matmuls.

Self-contained: hardcodes N=100000, DL=256, F=100, NFRAG=1000, 8 cores.
"""

import os

os.environ.setdefault("NEURON_RT_RESET_CORES", "1")  # recover wedged cores

import numpy as np

import concourse.bass as bass
import concourse.bacc as bacc
import concourse.tile as tile
import concourse.mybir as mybir
from concourse import bass_utils

F32 = mybir.dt.float32
F32R = mybir.dt.float32r
BF16 = mybir.dt.bfloat16
ALU = mybir.AluOpType
AFT = mybir.ActivationFunctionType
AXL = mybir.AxisListType

D = 3.8
N_ITER = 10
CONVERGED = 1e-4
MAX_LS = 10
NCAND = MAX_LS + 2  # 12: alpha*2^-k, k=0..10 evaluated, 2^-11 fallback


def build_program(ncores, fpc, F, DL, a2=None, niter=N_ITER, use_cc=True):
    """Build (unscheduled) Bacc program for one core (SPMD across ncores).

    a2: optional 3x3 np array baked into the program as immediates (the
    dgx mixing matrix); falls back to the a2rep input tile when None.
    """
    E = F - 1
    NPC = fpc * F
    d2 = float(np.float32(D * D))  # match reference: jnp.float32(D*D)
    hch = [(s, min(128, DL - s)) for s in range(0, DL, 128)]
    nh = len(hch)
    hdim = hch[0][1]
    NE = NCAND - 1  # 11 evaluated candidates

    nc = bacc.Bacc("TRN2", target_bir_lowering=False, debug=False,
                   enable_asserts=True, num_devices=ncores)

    y_in = nc.dram_tensor("y", [NPC, DL], F32R, kind="ExternalInput")
    wp3t_in = nc.dram_tensor("wp3t", [hdim, 3 * nh], F32R,
                             kind="ExternalInput")
    ident_in = nc.dram_tensor("ident", [fpc, fpc], F32R,
                              kind="ExternalInput")
    weffnt_in = nc.dram_tensor("weffnegt", [3, DL], F32R,
                               kind="ExternalInput")
    a2rep_in = nc.dram_tensor("a2rep", [fpc, 9], F32, kind="ExternalInput")
    onescol_in = nc.dram_tensor("onescol", [fpc, 1], F32, kind="ExternalInput")
    onesg_in = nc.dram_tensor("onesg", [ncores, 1], F32, kind="ExternalInput")
    onesrow_in = nc.dram_tensor("onesrow", [1, fpc], F32, kind="ExternalInput")
    pw2e_in = nc.dram_tensor("pw2e", [1, NCAND], F32, kind="ExternalInput")
    pw2sel_in = nc.dram_tensor("pw2sel", [1, NCAND], F32,
                               kind="ExternalInput")
    kvec_in = nc.dram_tensor("kvec", [1, 5], F32, kind="ExternalInput")
    k4_in = nc.dram_tensor("k4", [1, 4], F32, kind="ExternalInput")
    wb_in = nc.dram_tensor("wb", [1, 6], F32, kind="ExternalInput")
    yout = nc.dram_tensor("yout", [NPC, DL], F32, kind="ExternalOutput")

    with tile.TileContext(nc) as tc:
        with tc.tile_pool(name="persist", bufs=1) as P1, \
             tc.tile_pool(name="dram", bufs=1, space="DRAM") as PD:

            # ---------------- constants into SBUF ----------------
            def const_tile(shape, src, tag, dtype=F32):
                t = P1.tile(shape, dtype, tag=tag)
                nc.sync.dma_start(t[:], src.ap())
                return t
            wp3t = const_tile([hdim, 3 * nh], wp3t_in, "wp3t", F32R)
            ident = const_tile([fpc, fpc], ident_in, "ident", F32R)
            weffnt = const_tile([3, DL], weffnt_in, "weffnt", F32R)
            a2s = const_tile([fpc, 9], a2rep_in, "a2s")
            onescol = const_tile([fpc, 1], onescol_in, "onescol")
            onesg = const_tile([ncores, 1], onesg_in, "onesg")
            onesrow = const_tile([1, fpc], onesrow_in, "onesrow")
            pw2e = const_tile([1, NCAND], pw2e_in, "pw2e")
            pw2sel = const_tile([1, NCAND], pw2sel_in, "pw2sel")
            kvec = const_tile([1, 5], kvec_in, "kvec")
            k4 = const_tile([1, 4], k4_in, "k4")
            wb = const_tile([1, 6], wb_in, "wb")

            # warm up the collective path: first AllGather pays ~55us
            if use_cc:
                with tc.tile_pool(name="warmcc", bufs=1, space="DRAM") as PW:
                    win = PW.tile([1, 4], F32, tag="win")
                    wout = PW.tile([ncores, 4], F32, tag="wout")
                    wsb = P1.tile([1, 4], F32, tag="wsb")
                    nc.vector.memset(wsb[:], 0.0)
                    nc.sync.dma_start(win[:], wsb[:])
                    nc.gpsimd.collective_compute(
                        "AllGather", ALU.bypass,
                        replica_groups=[list(range(ncores))],
                        ins=[win[:].opt()], outs=[wout[:].opt()])
                    win2 = PW.tile([1, 4], F32, tag="win2")
                    wout2 = PW.tile([ncores, 4], F32, tag="wout2")
                    nc.sync.dma_start(win2[:], wsb[:])
                    nc.gpsimd.collective_compute(
                        "AllGather", ALU.bypass,
                        replica_groups=[list(range(ncores))],
                        ins=[win2[:].opt()], outs=[wout2[:].opt()])

            # warm the ACT sqrt table set early (overlaps DMA)
            warm = P1.tile([1, 1], F32)
            nc.vector.memset(warm[:], 1.0)
            nc.scalar.activation(warm[:], warm[:], AFT.Sqrt)

            # -------- y load (4 sub-tiles, SWDGE = all 16 engines) --------
            if F % 20 == 0:
                ybnd = [0, F // 10, F // 5, 3 * F // 5, F]
            else:
                ybnd = [0, F // 2, F]
            y_ap = y_in.ap().rearrange("(p i) d -> p (i d)", p=fpc)
            ynat = []
            for ci in range(len(ybnd) - 1):
                lo_i, hi_i = ybnd[ci], ybnd[ci + 1]
                t = P1.tile([fpc, (hi_i - lo_i) * DL], F32R, tag=f"ynat{ci}")
                nc.gpsimd.dma_start(
                    t[:], y_ap[:, lo_i * DL:hi_i * DL])
                ynat.append(t)

            def ynat_slice(i, lo, w):
                for ci in range(len(ybnd) - 1):
                    if i < ybnd[ci + 1]:
                        off = (i - ybnd[ci]) * DL + lo
                        return ynat[ci][:, off:off + w]
                raise AssertionError

            def ynat_slice2(i):
                # [fpc, 2*DL] covering nodes i, i+1 (must be in same region)
                for ci in range(len(ybnd) - 1):
                    if i < ybnd[ci + 1]:
                        assert i + 1 < ybnd[ci + 1] or ci == len(ybnd) - 2
                        assert i + 1 <= ybnd[ci + 1]
                        off = (i - ybnd[ci]) * DL
                        return ynat[ci][:, off:off + 2 * DL]
                raise AssertionError

            # ---------------- loop-state tiles ----------------
            dx = P1.tile([fpc, 3 * E], F32)       # dx planes [fpc,3,E]
            cqr = P1.tile([fpc, 3 * E], F32)      # [c | q | r]
            c_t = cqr[:, 0:E]
            q_t = cqr[:, E:2 * E]
            r_t = cqr[:, 2 * E:3 * E]
            cdxp = P1.tile([fpc, 3 * (F + 1)], F32)   # padded [fpc,3,F+1]
            lam = P1.tile([fpc, 3 * F], F32)
            dlam = P1.tile([fpc, 3 * E], F32)
            dgx = P1.tile([fpc, 3 * E], F32)
            qp = P1.tile([fpc, 3 * E], F32)
            prod5 = P1.tile([fpc, 5 * E], F32)
            S_t = P1.tile([fpc, 3 * F], F32)
            scr2 = P1.tile([fpc, F], F32)
            stage_t = P1.tile([fpc, 16], F32)
            g8row = P1.tile([1, ncores * 16], F32)
            x3T = P1.tile([3, NPC], F32, tag="big3")
            x3p = P1.tile([fpc, 3 * F], F32)

            # scalar-stage tiles (partition 0)
            alpha = P1.tile([1, 1], F32)
            done = P1.tile([1, 1], F32)
            S0 = P1.tile([1, 1], F32)
            av = P1.tile([1, NCAND], F32)
            av2 = P1.tile([1, NCAND], F32)
            av3 = P1.tile([1, NCAND], F32)
            av4 = P1.tile([1, NCAND], F32)
            avsel = P1.tile([1, NCAND], F32)
            t5 = P1.tile([1, 5], F32)
            ct = P1.tile([1, NCAND], F32)
            okx = P1.tile([1, NCAND], F32)
            t12 = P1.tile([1, NCAND], F32)
            selv = P1.tile([1, NCAND], F32)
            selc = P1.tile([1, NCAND], F32)
            af = P1.tile([1, 1], F32)
            ctf = P1.tile([1, 1], F32)
            sA = P1.tile([1, 1], F32)
            sB = P1.tile([1, 1], F32)
            sC = P1.tile([1, 1], F32)
            u_t = P1.tile([1, 1], F32)
            sreg = P1.tile([1, 4], F32)
            pgs = P1.tile([1, 16], F32)
            s16 = P1.tile([1, 16], F32)
            bs_sb = P1.tile([fpc, 4], F32)
            S_scr = P1.tile([fpc, 3 * F], F32)
            sGA = P1.tile([1, 1], F32)
            sGB = P1.tile([1, 1], F32)
            sD = P1.tile([1, 1], F32)
            one1 = P1.tile([1, 1], F32)
            convc = P1.tile([1, 1], F32)
            Sbf = P1.tile([fpc, 3 * F], BF16)
            weffbf = P1.tile([3, DL], BF16)

            nc.vector.memset(cdxp[:], 0.0)
            nc.vector.memset(S_t[:], 0.0)
            nc.vector.memset(done[:], 0.0)
            nc.vector.memset(u_t[:], 1.0)
            nc.vector.memset(one1[:], 1.0)
            nc.vector.memset(convc[:], CONVERGED)
            nc.vector.memset(stage_t[:], 0.0)
            nc.vector.memset(okx[:, NCAND - 1:NCAND], 1.0)

            # 3d views
            dx3 = dx[:].rearrange("p (c e) -> p c e", c=3)
            cdxp3 = cdxp[:].rearrange("p (c e) -> p c e", c=3)
            lam3 = lam[:].rearrange("p (c e) -> p c e", c=3)
            dlam3 = dlam[:].rearrange("p (c e) -> p c e", c=3)
            dgx3 = dgx[:].rearrange("p (c e) -> p c e", c=3)
            qp3 = qp[:].rearrange("p (c e) -> p c e", c=3)
            x3p3 = x3p[:].rearrange("p (c e) -> p c e", c=3)
            x3T3 = x3T[:].rearrange("c (p f) -> c p f", f=F)

            # ---------------- Phase A: x3 = y @ Wp3.T ----------------
            # fp32r transposes (1.5 c/r) + 512-wide fp32r matmuls (1 c/r).
            IB = 4 if F % 4 == 0 else 2  # i's per psum bank
            with tc.tile_pool(name="psA", bufs=4, space="PSUM") as PSA, \
                 tc.tile_pool(name="psX", bufs=2, space="PSUM") as PSX, \
                 tc.tile_pool(name="yt", bufs=3) as PYT:
                for g in range(F // IB):
                    psx = PSX.tile([3, IB * fpc], F32, tag="psx")
                    yt4 = PYT.tile([hdim, nh * IB * fpc], F32R, tag="yt4")
                    yt4v = yt4[:].rearrange("p (h f) -> p h f", h=nh)
                    for i2 in range(IB):
                        i = g * IB + i2
                        pst = PSA.tile([hdim, nh * fpc], F32R, tag="pst")
                        for h, (lo, w) in enumerate(hch):
                            nc.tensor.transpose(
                                pst[:w, h * fpc:(h + 1) * fpc],
                                ynat_slice(i, lo, w),
                                ident[:])
                        dst = yt4v[:, :, i2 * fpc:(i2 + 1) * fpc]
                        src = pst[:].rearrange("p (h f) -> p h f", h=nh)
                        nc.vector.tensor_copy(dst, src)
                    for h in range(nh):
                        nc.tensor.matmul(
                            psx[:],
                            lhsT=wp3t[:, 3 * h:3 * h + 3],
                            rhs=yt4[:, h * IB * fpc:(h + 1) * IB * fpc],
                            start=(h == 0), stop=(h == nh - 1))
                    # drain bank -> x3T columns {p*F + g*IB + i2} (ACT)
                    src = psx[:].rearrange("c (d p) -> c p d", p=fpc)
                    dst = x3T3[:, :, g * IB:(g + 1) * IB]
                    nc.scalar.activation(dst, src, AFT.Copy)

            # scatter x3T -> fragment-major planes
            for j in range(3):
                nc.sync.dma_start(x3p[:, j * F:(j + 1) * F], x3T[j:j + 1, :])
            # dx = diff(x3) ; c = |dx|^2 - d2
            nc.vector.tensor_tensor(out=dx3[:, :, :], in0=x3p3[:, :, 1:F],
                                    in1=x3p3[:, :, 0:E], op=ALU.subtract)
            nc.vector.tensor_tensor(out=qp[:], in0=dx[:], in1=dx[:],
                                    op=ALU.mult)
            nc.vector.tensor_tensor(out=c_t, in0=qp3[:, 0, :],
                                    in1=qp3[:, 1, :], op=ALU.add)
            nc.vector.tensor_tensor(out=c_t, in0=c_t, in1=qp3[:, 2, :],
                                    op=ALU.add)
            nc.vector.tensor_scalar_add(out=c_t, in0=c_t, scalar1=-d2)

            # ---------------- Phase B: constraint iterations ----------------
            with tc.tile_pool(name="psS", bufs=1, space="PSUM") as PSS, \
                 tc.tile_pool(name="psB", bufs=1, space="PSUM") as PSB, \
                 tc.tile_pool(name="ccd", bufs=2, space="DRAM") as PCD:

                for it in range(niter):
                    first = (it == 0)
                    # cdx = c * dx  (into padded buffer cols 1..F-1)
                    nc.vector.tensor_tensor(
                        out=cdxp3[:, :, 1:F], in0=dx3[:, :, :],
                        in1=c_t.unsqueeze(1).broadcast_to((fpc, 3, E)),
                        op=ALU.mult)
                    # dlam[e] = 2*cdxp[e+1] - cdxp[e] - cdxp[e+2]
                    nc.vector.scalar_tensor_tensor(
                        out=dlam3[:, :, :], in0=cdxp3[:, :, 1:F], scalar=2.0,
                        in1=cdxp3[:, :, 0:E], op0=ALU.mult, op1=ALU.subtract)
                    nc.vector.tensor_tensor(
                        out=dlam3[:, :, :], in0=dlam3[:, :, :],
                        in1=cdxp3[:, :, 2:F + 1], op=ALU.subtract)
                    if first:  # lam needed pre-AG only for the lamB sums
                        nc.vector.tensor_tensor(
                            out=lam3[:, :, :], in0=cdxp3[:, :, 0:F],
                            in1=cdxp3[:, :, 1:F + 1], op=ALU.subtract)
                    # dgx_j = sum_k A2[j,k] dlam_k
                    for j in range(3):
                        if a2 is not None:  # immediates: no SBUF ptr reads
                            nc.vector.tensor_scalar_mul(
                                out=dgx3[:, j, :], in0=dlam3[:, 0, :],
                                scalar1=float(a2[j, 0]))
                            for k in (1, 2):
                                nc.vector.scalar_tensor_tensor(
                                    out=dgx3[:, j, :], in0=dlam3[:, k, :],
                                    scalar=float(a2[j, k]),
                                    in1=dgx3[:, j, :],
                                    op0=ALU.mult, op1=ALU.add)
                        else:
                            nc.vector.tensor_scalar_mul(
                                out=dgx3[:, j, :], in0=dlam3[:, 0, :],
                                scalar1=a2s[:, 3 * j:3 * j + 1])
                            for k in (1, 2):
                                nc.vector.scalar_tensor_tensor(
                                    out=dgx3[:, j, :], in0=dlam3[:, k, :],
                                    scalar=a2s[:, 3 * j + k:3 * j + k + 1],
                                    in1=dgx3[:, j, :],
                                    op0=ALU.mult, op1=ALU.add)
                    # q = <dx, dgx>, r = <dgx, dgx>
                    nc.vector.tensor_tensor(out=qp[:], in0=dx[:], in1=dgx[:],
                                            op=ALU.mult)
                    nc.vector.tensor_tensor(out=q_t, in0=qp3[:, 0, :],
                                            in1=qp3[:, 1, :], op=ALU.add)
                    nc.vector.tensor_tensor(out=q_t, in0=q_t,
                                            in1=qp3[:, 2, :], op=ALU.add)
                    nc.vector.tensor_tensor(out=qp[:], in0=dgx[:], in1=dgx[:],
                                            op=ALU.mult)
                    nc.vector.tensor_tensor(out=r_t, in0=qp3[:, 0, :],
                                            in1=qp3[:, 1, :], op=ALU.add)
                    nc.vector.tensor_tensor(out=r_t, in0=r_t,
                                            in1=qp3[:, 2, :], op=ALU.add)

                    # batched partial sums: prod5 = [cq, qr, q2, r2, cr]
                    nc.vector.tensor_tensor(out=prod5[:, 0:2 * E],
                                            in0=cqr[:, 0:2 * E],
                                            in1=cqr[:, E:3 * E], op=ALU.mult)
                    nc.vector.tensor_tensor(out=prod5[:, 2 * E:4 * E],
                                            in0=cqr[:, E:3 * E],
                                            in1=cqr[:, E:3 * E], op=ALU.mult)
                    nc.vector.tensor_tensor(out=prod5[:, 4 * E:5 * E],
                                            in0=c_t, in1=r_t, op=ALU.mult)
                    nc.vector.tensor_reduce(
                        out=stage_t[:, 0:5],
                        in_=prod5[:].rearrange("p (g e) -> p g e", g=5),
                        axis=AXL.X, op=ALU.add)
                    if first:
                        nc.vector.tensor_tensor(out=prod5[:, 0:E], in0=c_t,
                                                in1=c_t, op=ALU.mult)
                        nc.vector.tensor_reduce(
                            out=stage_t[:, 5:6], in_=prod5[:, 0:E],
                            axis=AXL.X, op=ALU.add)
                        for col, (ja, jb) in enumerate(
                                [(0, 0), (1, 1), (2, 2), (0, 1), (0, 2),
                                 (1, 2)]):
                            nc.vector.tensor_tensor(
                                out=scr2[:], in0=lam3[:, ja, :],
                                in1=lam3[:, jb, :], op=ALU.mult)
                            nc.vector.tensor_reduce(
                                out=stage_t[:, 6 + col:7 + col], in_=scr2[:],
                                axis=AXL.X, op=ALU.add)

                    # partition-reduce on PE, bounce, allgather.  Issue the
                    # collective FIRST, then the shadow ops — anything queued
                    # ahead of the collective on the gpsimd/DVE queues delays
                    # its trigger by that op's execution time.
                    ps1 = PSS.tile([1, 16], F32, tag="ps1")
                    nc.tensor.matmul(ps1[:], lhsT=onescol[:], rhs=stage_t[:],
                                     start=True, stop=True)
                    nc.vector.tensor_copy(s16[:], ps1[:])
                    cc_in = PCD.tile([1, 16], F32, tag="cci")
                    cc_out = PCD.tile([ncores, 16], F32, tag="cco")
                    nc.sync.dma_start(cc_in[:], s16[:])
                    if use_cc:
                        nc.gpsimd.collective_compute(
                            "AllGather", ALU.bypass,
                            replica_groups=[list(range(ncores))],
                            ins=[cc_in[:].opt()], outs=[cc_out[:].opt()])
                    else:  # debug: replicate local sums
                        nc.sync.dma_start(
                            cc_out[:],
                            cc_in[0:1, :].broadcast_to((ncores, 16)))
                    nc.sync.dma_start(g8row[:], cc_out[:])

                    # hole-fillers: run during the collective round-trip
                    if not first:
                        nc.gpsimd.tensor_tensor(
                            out=lam3[:, :, :], in0=cdxp3[:, :, 0:F],
                            in1=cdxp3[:, :, 1:F + 1], op=ALU.subtract)
                    if not first:  # iter 0: alpha not yet known
                        nc.vector.tensor_scalar_mul(out=av[:], in0=pw2e[:],
                                                    scalar1=alpha[:])
                        nc.vector.tensor_scalar_mul(out=avsel[:],
                                                    in0=pw2sel[:],
                                                    scalar1=alpha[:])
                        nc.vector.tensor_tensor(out=av2[:], in0=av[:],
                                                in1=av[:], op=ALU.mult)
                        nc.vector.tensor_tensor(out=av3[:], in0=av2[:],
                                                in1=av[:], op=ALU.mult)
                        nc.vector.tensor_tensor(out=av4[:], in0=av2[:],
                                                in1=av2[:], op=ALU.mult)
                        # u = 1 - done (prev iteration's done)
                        nc.gpsimd.tensor_tensor(out=u_t[:], in0=one1[:],
                                                in1=done[:], op=ALU.subtract)

                    nc.vector.tensor_reduce(
                        out=pgs[:],
                        in_=g8row[:].rearrange("o (r c) -> o c r", c=16),
                        axis=AXL.X, op=ALU.add)

                    # ---------------- scalar stage ----------------
                    if first:
                        # alpha = 1/sqrt(dot(wb, pgs[6:12])), newton-polished
                        nc.vector.tensor_tensor(out=t12[:, 0:6],
                                                in0=pgs[:, 6:12], in1=wb[:],
                                                op=ALU.mult)
                        nc.vector.tensor_reduce(out=sA[:], in_=t12[:, 0:6],
                                                axis=AXL.X, op=ALU.add)
                        nc.scalar.activation(sB[:], sA[:], AFT.Sqrt)
                        nc.vector.reciprocal(alpha[:], sB[:])
                        nc.vector.tensor_tensor(out=sB[:], in0=alpha[:],
                                                in1=alpha[:], op=ALU.mult)
                        nc.vector.tensor_scalar(out=sB[:], in0=sB[:],
                                                scalar1=sA[:], scalar2=-0.5,
                                                op0=ALU.mult, op1=ALU.mult)
                        nc.vector.tensor_scalar_add(out=sB[:], in0=sB[:],
                                                    scalar1=1.5)
                        nc.vector.tensor_tensor(out=alpha[:], in0=alpha[:],
                                                in1=sB[:], op=ALU.mult)
                        nc.vector.tensor_copy(S0[:], pgs[:, 5:6])
                        nc.vector.tensor_scalar_mul(out=av[:], in0=pw2e[:],
                                                    scalar1=alpha[:])
                        nc.vector.tensor_scalar_mul(out=avsel[:],
                                                    in0=pw2sel[:],
                                                    scalar1=alpha[:])
                        nc.vector.tensor_tensor(out=av2[:], in0=av[:],
                                                in1=av[:], op=ALU.mult)
                        nc.vector.tensor_tensor(out=av3[:], in0=av2[:],
                                                in1=av[:], op=ALU.mult)
                        nc.vector.tensor_tensor(out=av4[:], in0=av2[:],
                                                in1=av2[:], op=ALU.mult)
                    # t5 = kvec * [Scq, Sqr, Sq2, Sr2, Scr]
                    #    = [-4S1, -4S4, 4S2, S5, 2S3]
                    nc.vector.tensor_tensor(out=t5[:], in0=pgs[:, 0:5],
                                            in1=kvec[:], op=ALU.mult)
                    nc.vector.scalar_tensor_tensor(
                        out=ct[:], in0=av[:], scalar=t5[:, 0:1],
                        in1=S0[:].broadcast_to((1, NCAND)),
                        op0=ALU.mult, op1=ALU.add)
                    for pw, colx in ((av2, 2), (av2, 4), (av3, 1), (av4, 3)):
                        nc.vector.scalar_tensor_tensor(
                            out=ct[:], in0=pw[:],
                            scalar=t5[:, colx:colx + 1], in1=ct[:],
                            op0=ALU.mult, op1=ALU.add)
                    # ok[k] = ct[k] < cnorm ; col NCAND-1 preset to 1
                    nc.vector.tensor_scalar(out=okx[:, 0:NE], in0=ct[:, 0:NE],
                                            scalar1=S0[:], scalar2=None,
                                            op0=ALU.is_lt)
                    # first success (av strictly decreasing): af = max(ok*av)
                    nc.vector.tensor_tensor(out=t12[:], in0=okx[:],
                                            in1=avsel[:], op=ALU.mult)
                    nc.vector.tensor_reduce(out=af[:], in_=t12[:],
                                            axis=AXL.X, op=ALU.max)
                    # ctf = ct at the selected candidate (unique max match)
                    nc.vector.tensor_scalar(out=selv[:], in0=t12[:],
                                            scalar1=af[:], scalar2=None,
                                            op0=ALU.is_equal)
                    nc.vector.tensor_tensor(out=selc[:], in0=selv[:],
                                            in1=ct[:], op=ALU.mult)
                    nc.vector.tensor_reduce(out=ctf[:], in_=selc[:],
                                            axis=AXL.X, op=ALU.add)
                    # growth: af *= 1 + 0.5*ok0*(ctf > CONVERGED)
                    nc.vector.tensor_scalar(out=sA[:], in0=ctf[:],
                                            scalar1=CONVERGED, scalar2=None,
                                            op0=ALU.is_gt)
                    nc.vector.tensor_scalar(out=sA[:], in0=sA[:],
                                            scalar1=okx[:, 0:1], scalar2=0.5,
                                            op0=ALU.mult, op1=ALU.mult)
                    nc.vector.scalar_tensor_tensor(
                        out=af[:], in0=sA[:], scalar=1.0, in1=af[:],
                        op0=ALU.add, op1=ALU.mult)
                    # a_eff = af*u  (u = 1-done from prev iteration)
                    nc.vector.tensor_tensor(out=sC[:], in0=af[:], in1=u_t[:],
                                            op=ALU.mult)
                    # sreg = [-a, -2a, a^2, 2a]
                    nc.vector.tensor_scalar_mul(out=sreg[:], in0=k4[:],
                                                scalar1=sC[:])
                    nc.vector.tensor_tensor(out=sreg[:, 2:3], in0=sC[:],
                                            in1=sC[:], op=ALU.mult)

                    # broadcast sreg via PE, apply updates
                    bs = PSB.tile([fpc, 4], F32, tag="bs")
                    nc.tensor.matmul(bs[:], lhsT=onesrow[:], rhs=sreg[:],
                                     start=True, stop=True)
                    nc.vector.scalar_tensor_tensor(
                        out=dx[:], in0=dgx[:], scalar=bs[:, 0:1], in1=dx[:],
                        op0=ALU.mult, op1=ALU.add)
                    nc.vector.scalar_tensor_tensor(
                        out=c_t, in0=q_t, scalar=bs[:, 1:2], in1=c_t,
                        op0=ALU.mult, op1=ALU.add)
                    nc.vector.scalar_tensor_tensor(
                        out=c_t, in0=r_t, scalar=bs[:, 2:3], in1=c_t,
                        op0=ALU.mult, op1=ALU.add)
                    nc.scalar.activation(bs_sb[:], bs[:], AFT.Copy)
                    nc.gpsimd.tensor_tensor(
                        out=S_scr[:], in0=lam[:],
                        in1=bs_sb[:, 3:4].broadcast_to((fpc, 3 * F)),
                        op=ALU.mult)
                    nc.gpsimd.tensor_tensor(
                        out=S_t[:], in0=S_t[:], in1=S_scr[:], op=ALU.add)

                    # state upkeep on GpSimd, off the critical path (consumed
                    # only by the NEXT iteration's shadow / scalar stage)
                    nc.gpsimd.tensor_tensor(out=sGA[:], in0=af[:],
                                            in1=alpha[:], op=ALU.subtract)
                    nc.gpsimd.tensor_tensor(out=sGA[:], in0=sGA[:],
                                            in1=u_t[:], op=ALU.mult)
                    nc.gpsimd.tensor_tensor(out=alpha[:], in0=alpha[:],
                                            in1=sGA[:], op=ALU.add)
                    nc.gpsimd.tensor_tensor(out=sGB[:], in0=ctf[:],
                                            in1=S0[:], op=ALU.subtract)
                    nc.gpsimd.tensor_tensor(out=sGB[:], in0=sGB[:],
                                            in1=u_t[:], op=ALU.mult)
                    nc.gpsimd.tensor_tensor(out=S0[:], in0=S0[:],
                                            in1=sGB[:], op=ALU.add)
                    nc.vector.tensor_tensor(out=sD[:], in0=ctf[:],
                                            in1=convc[:], op=ALU.is_lt)
                    nc.vector.tensor_tensor(out=done[:], in0=done[:],
                                            in1=sD[:], op=ALU.max)

            # ---------------- Phase C: yout = y - S @ Weff.T ----------------
            # bf16 matmuls: 1 cycle/row at any p-state + fast weight load
            nc.vector.tensor_copy(weffbf[:], weffnt[:].bitcast(F32))
            nc.vector.tensor_copy(Sbf[:], S_t[:])
            ST = x3T[:, 0:NPC // 2].bitcast(BF16)  # reuse x3T's bytes
            S3 = Sbf[:].rearrange("p (c f) -> p c f", c=3)
            for j in range(3):
                nc.sync.dma_start(ST[j:j + 1, :], S3[:, j, :])
            ST3 = ST.rearrange("c (p f) -> c p f", f=F)
            OB = 10 if F % 10 == 0 else max(
                b for b in (5, 2, 1) if F % b == 0)  # i's per out block
            dst_y = yout.ap().rearrange("(p f) d -> p f d", p=fpc)
            with tc.tile_pool(name="psF", bufs=4, space="PSUM") as PSF, \
                 tc.tile_pool(name="obuf", bufs=2) as POB:
                for blk in range(F // OB):
                    ob = POB.tile([fpc, OB * DL], F32, tag="ob")
                    for i2 in range(0, OB, 2):
                        i = blk * OB + i2
                        bank = PSF.tile([fpc, 2 * DL], F32, tag="fin")
                        for t in range(2):
                            nc.tensor.matmul(
                                bank[:, t * DL:(t + 1) * DL],
                                lhsT=ST3[:, :, i + t],
                                rhs=weffbf[:],
                                start=True, stop=True)
                        pair_dst = ob[:, i2 * DL:(i2 + 2) * DL]
                        if (i2 // 2) % 5 != 2:  # 4/5 of pairs on DVE
                            nc.vector.tensor_tensor(
                                out=pair_dst, in0=bank[:],
                                in1=ynat_slice2(i).bitcast(F32), op=ALU.add)
                        else:  # 1/5 via ACT copy + GpSimd add
                            sc = POB.tile([fpc, 2 * DL], F32, tag="sc")
                            nc.scalar.activation(sc[:], bank[:], AFT.Copy)
                            nc.gpsimd.tensor_tensor(
                                out=pair_dst, in0=sc[:],
                                in1=ynat_slice2(i).bitcast(F32), op=ALU.add)
                    nc.gpsimd.dma_start(
                        dst_y[:, blk * OB:(blk + 1) * OB, :], ob[:])

    return nc


def make_consts(Wp, Wu, fpc, ncores, nreal=None):
    if nreal is None:
        nreal = fpc
    DL = Wp.shape[1]
    hch = [(s, min(128, DL - s)) for s in range(0, DL, 128)]
    nh = len(hch)
    hdim = hch[0][1]
    Wp3 = Wp[:3].astype(np.float32)
    Weff = (Wu[:, 0:3] + Wu[:, 3:6] + Wu[:, 6:9]).astype(np.float32)
    A2 = 2.0 * (Wp3 @ Weff)
    B = Weff.T @ Weff
    wb = 4.0 * np.array([[B[0, 0], B[1, 1], B[2, 2],
                          2 * B[0, 1], 2 * B[0, 2], 2 * B[1, 2]]], np.float32)
    wp3t = np.zeros((hdim, 3 * nh), np.float32)
    for h, (lo, w) in enumerate(hch):
        wp3t[:w, 3 * h:3 * h + 3] = Wp3[:, lo:lo + w].T
    pw2e = (2.0 ** -np.arange(NCAND, dtype=np.float32))
    pw2e[NCAND - 1] = pw2e[NCAND - 2]  # last candidate duplicates ct(2^-10)
    pw2sel = (2.0 ** -np.arange(NCAND, dtype=np.float32))
    return {
        "wp3t": wp3t,
        "ident": np.eye(fpc, dtype=np.float32),
        "weffnegt": np.ascontiguousarray(-Weff.T),
        "a2rep": np.tile(A2.reshape(1, 9), (fpc, 1)).astype(np.float32),
        "onescol": (np.arange(fpc) < nreal).astype(
            np.float32).reshape(fpc, 1),
        "onesg": np.ones((ncores, 1), np.float32),
        "onesrow": np.ones((1, fpc), np.float32),
        "pw2e": pw2e.reshape(1, NCAND),
        "pw2sel": pw2sel.reshape(1, NCAND),
        "kvec": np.array([[-4.0, -4.0, 4.0, 1.0, 2.0]], np.float32),
        "k4": np.array([[-1.0, -2.0, 0.0, 2.0]], np.float32),
        "wb": wb,
    }


_PROG_CACHE = {}


def _get_program(ncores, fpc, F, DL, a2):
    key = (ncores, fpc, F, DL, a2.tobytes())
    if key not in _PROG_CACHE:
        nc = build_program(ncores, fpc, F, DL, a2=a2)
        nc.compile()
        _PROG_CACHE[key] = nc
    return _PROG_CACHE[key]


def prepare(inputs):
    """Build/compile program and padded in_maps (shared with test harness)."""
    y = np.ascontiguousarray(np.asarray(inputs["y"], np.float32))
    Wp = np.asarray(inputs["Wp"], np.float32)
    Wu = np.asarray(inputs["Wu"], np.float32)
    N, DL = y.shape
    NCORES, F = 8, 100
    fpc = N // F // NCORES
    NPC = N // NCORES
    fpc_pad = 128
    NPC_pad = fpc_pad * F
    Wp3 = Wp[:3].astype(np.float32)
    Weff = (Wu[:, 0:3] + Wu[:, 3:6] + Wu[:, 6:9]).astype(np.float32)
    a2 = (2.0 * (Wp3 @ Weff)).astype(np.float32)
    nc = _get_program(NCORES, fpc_pad, F, DL, a2)
    consts = make_consts(Wp, Wu, fpc_pad, NCORES, nreal=fpc)
    in_maps = []
    for i in range(NCORES):
        sh = np.zeros((NPC_pad, DL), np.float32)
        sh[:NPC] = y[i * NPC:(i + 1) * NPC]
        in_maps.append({"y": sh, **consts})
    return nc, in_maps, NPC


def kernel(**inputs):
    y = np.ascontiguousarray(np.asarray(inputs["y"], np.float32))
    Wp = np.asarray(inputs["Wp"], np.float32)
    Wu = np.asarray(inputs["Wu"], np.float32)
    N, DL = y.shape
    NCORES, F = 8, 100
    NFRAG = N // F
    fpc = NFRAG // NCORES
    NPC = N // NCORES

    nc, in_maps, NPC_r = prepare(inputs)
    res = bass_utils.run_bass_kernel_spmd(
        nc, in_maps, core_ids=list(range(NCORES)))
    out = np.concatenate(
        [res.results[i]["yout"][:NPC_r] for i in range(NCORES)], axis=0)
    return out.astype(inputs["y"].dtype, copy=False)
